# revision 6
# baseline (speedup 1.0000x reference)
"""Gumbel-softmax palette quantization on 8 TRN2 NeuronCores.

Math (per batch b, pixel p, palette entry k), T = temperature:
    gumbel = -ln(-ln(u + eps))
    probs  = softmax((img + gumbel) / T, axis=k)
    out    = probs @ palette                      # [pix, 4]

Fast path (T == 1): softmax is scale-invariant per pixel, and
    exp(img + gumbel) = exp(img) * (-ln u)^(-1) ,
so one of the two Ln passes of the naive form can be replaced by a
reciprocal that runs on the Vector engine instead of the (bottleneck)
Scalar engine.  Per element the device computes
    L  = ln(v + beta)          # ACT Ln;  v = u-1 shipped fp16, beta = 1-1e-5
    E  = exp(img / T)          # ACT Exp (in-place on the img tile)
    et = E * approx(1/L)       # one fused custom-DVE op (seed + 1 NR + mul)
    out[p, c] = (et @ pal_aug)[p, c] / (et @ pal_aug)[p, 4]
with pal_aug = [palette | ones].  et is negative everywhere (L < 0); the
sign cancels in the numerator/denominator ratio.  Shipping v = u-1 keeps
full precision where it matters (u -> 1 == gumbel winners); beta < 1 caps
|1/L| at ~1e5 which is far above any competing weight, so the cap is
invisible in the output.

Layout: inputs are pre-transposed ON THE HOST to k-major [k, pix] fp16 so
no on-device transpose is needed: the contraction axis k is already on
partitions and the palette matmul consumes et tiles directly as lhsT.
Halving input bytes (fp16) also halves HBM traffic.

Sharding: data-parallel over batch, 1 batch per core (b=8, 8 cores).
"""

import numpy as np
import ml_dtypes

B, H, W, K, C = 8, 256, 256, 256, 4
NPIX = H * W                # 65536 pixels per batch/core
F = 4096                    # pixels per block
NBLK = NPIX // F            # 16 blocks
JB = F // 128               # 32 pixel-groups of 128 per block
NHALF = K // 128            # 2 k-halves
EPS = 1e-20
NCORES = 8
BETA = 1.0 - 1e-5           # ln bias: caps |1/ln| at ~1e5, avoids ln(<=0)
VMIN = float(-BETA + 2e-3)  # host clamp for v = u-1 (u ~ 0 edge)
DIV_C0, DIV_C1 = -0.23549792, 2.0017324  # Chebyshev pair for the recip seed

_cache: dict = {}
_div_op = None


def _get_div_op():
    """Fused out = in1 * approx(1/in0) as a single custom-DVE op.

    Reuses RECIPROCAL_APPROX_FAST's BITWISE_NOT exponent-flip seed plus ONE
    inline Newton-Raphson pass (rel err <= ~1.8e-3, below the bf16 output
    rounding that follows), leaving pipeline stages for the * in1 multiply.
    Registered at runtime in a free custom-DVE opcode row.
    """
    global _div_op
    if _div_op is not None:
        return _div_op
    from concourse import dve_ops
    from concourse.dve_spec import (
        AluOp, Bin, Spec, Src0, Src1, C0 as SC0, C1 as SC1, lower, _has_src1,
    )
    from concourse.dve_uop import DveOpSpec
    from concourse.dve_table_gen import dve_ver_for, free_opcode_rows

    name = "DIV_APPROX_ANT"
    for op in dve_ops.OPS:
        if op.name == name:
            _div_op = op
            return op

    _not_x = Bin(AluOp.BITWISE_NOT, Src0, Src0)
    _y0 = _not_x * SC0
    body = Src1 * (_y0 * (SC1 - Src0 * _y0))

    def _ref(in0, in1, c0, c1, c2):
        not_x = (~in0.view(np.int32)).view(np.float32)
        y0 = not_x * c0
        return in1.astype(np.float32) * (y0 * (c1 - in0 * y0))

    spec = Spec(body=body, reference=_ref)
    used = set(dve_ops._SUB_OPCODE_FOR_NAME.values())
    row = None
    for r in free_opcode_rows("TRN2"):
        if r not in used:
            row = r
            break
    if row is None:  # fall back to the row of an op this kernel never emits
        row = dve_ops.get_dve_sub_opcode("ADD_RANGE_WRAP")
    dve_ops._SUB_OPCODE_FOR_NAME[name] = row
    ver = dve_ver_for("TRN2")
    sha = DveOpSpec(
        name=name, opcode=row, uops=lower(spec, ver=ver), rd1_en=_has_src1(spec)
    ).sha(ver)
    op = dve_ops.DveOp(name, spec, subdim=False, uops_sha={ver: sha})
    dve_ops.OPS.append(op)
    dve_ops.CUSTOM_DVE_SPECS[name] = spec
    _div_op = op
    return op


def _build(temp: float, repeat: int = 1):
    import concourse.mybir as mybir
    from concourse import bacc
    from concourse.tile import TileContext

    dt = mybir.dt
    AF = mybir.ActivationFunctionType
    divop = _get_div_op()

    nc = bacc.Bacc(
        "TRN2", target_bir_lowering=False, debug=False, num_devices=NCORES
    )

    img_d = nc.dram_tensor("imgt", [NHALF * NBLK, 128, F], dt.float16, kind="ExternalInput")
    v_d = nc.dram_tensor("vt", [NHALF * NBLK, 128, F], dt.float16, kind="ExternalInput")
    pal_d = nc.dram_tensor("pal", [128, NHALF, 5], dt.bfloat16, kind="ExternalInput")
    out_d = nc.dram_tensor("out", [NBLK, 128, JB * 4], dt.float32, kind="ExternalOutput")

    with TileContext(nc) as tc:
        with (
            tc.tile_pool(name="const", bufs=1) as cpool,
            tc.tile_pool(name="img", bufs=4) as ipool,
            tc.tile_pool(name="v", bufs=4) as vpool,
            tc.tile_pool(name="lt", bufs=2) as lpool,
            tc.tile_pool(name="et", bufs=4) as epool,
            tc.tile_pool(name="outp", bufs=3) as opool,
            tc.tile_pool(name="epi", bufs=2) as xpool,
            tc.tile_pool(name="acc", bufs=2, space="PSUM") as accpool,
        ):
            pal = cpool.tile([128, NHALF, 5], dt.bfloat16, tag="pal")
            nc.sync.dma_start(pal[:], pal_d[:])
            betab = cpool.tile([128, 1], dt.float32, tag="betab")
            nc.vector.memset(betab[:], BETA)

            for _rep in range(repeat):
                for bi in range(NBLK):
                    ets = []
                    for h in range(NHALF):
                        ui = h * NBLK + bi
                        img = ipool.tile([128, F], dt.float16)
                        v = vpool.tile([128, F], dt.float16)
                        nc.sync.dma_start(img[:], img_d[ui])
                        nc.sync.dma_start(v[:], v_d[ui])
                        L = lpool.tile([128, F], dt.float16)
                        nc.scalar.activation(L[:], v[:], AF.Ln, bias=betab[:])
                        et = epool.tile([128, F], dt.bfloat16)
                        if temp == 1.0:
                            nc.scalar.activation(img[:], img[:], AF.Exp, scale=1.0 / temp)
                            nc.vector._custom_dve(
                                divop, out=et[:], in0=L[:], in1=img[:],
                                s0=DIV_C0, s1=DIV_C1, imm2=0.0,
                            )
                        else:
                            # z-form: e = exp((img - ln(-L)) / T)
                            g = vpool.tile([128, F], dt.float16)
                            nc.scalar.activation(g[:], L[:], AF.Ln, scale=-1.0)
                            nc.vector.tensor_sub(img[:], img[:], g[:])
                            nc.scalar.activation(et[:], img[:], AF.Exp, scale=1.0 / temp)
                        ets.append(et)

                    acc = accpool.tile([128, JB * 5], dt.float32)
                    for j in range(JB):
                        for h in range(NHALF):
                            nc.tensor.matmul(
                                acc[:, j * 5:(j + 1) * 5],
                                ets[h][:, j * 128:(j + 1) * 128],
                                pal[:, h, :],
                                start=(h == 0),
                                stop=(h == NHALF - 1),
                            )

                    # epilogue: divide colors by the ones-column sum
                    av = acc[:].rearrange("p (j c) -> p j c", c=5)
                    sinv = xpool.tile([128, JB], dt.float32, tag="sinv")
                    nc.vector.reciprocal(sinv[:], av[:, :, 4])
                    outf = opool.tile([128, JB * 4], dt.float32)
                    ov = outf[:].rearrange("p (j c) -> p j c", c=4)
                    nc.vector.tensor_mul(
                        ov[:], av[:, :, 0:4],
                        sinv[:, :, None].broadcast_to([128, JB, 4]),
                    )
                    nc.sync.dma_start(out_d[bi], outf[:])

    nc.compile()
    return nc


def _get_nc(temp: float, repeat: int = 1):
    key = (temp, repeat)
    if key not in _cache:
        _cache[key] = _build(temp, repeat)
    return _cache[key]


def _khalf_blocks(a: np.ndarray) -> np.ndarray:
    """[NPIX, K] fp16 -> [NHALF*NBLK, 128, F], k-major with k on partitions."""
    # k = h*128 + r  <->  partition r of half h (matches the palette layout)
    t = a.T.reshape(NHALF, 128, NBLK, F)
    return np.ascontiguousarray(t.transpose(0, 2, 1, 3)).reshape(NHALF * NBLK, 128, F)


def _make_in_maps(images, palettes, uniform_noise):
    in_maps = []
    for i in range(NCORES):
        aug = np.concatenate(
            [palettes[i].astype(np.float32), np.ones((K, 1), np.float32)], axis=1
        )  # [256, 5]
        pal = np.ascontiguousarray(
            aug.reshape(NHALF, 128, 5).transpose(1, 0, 2)
        ).astype(ml_dtypes.bfloat16)  # [128(k_lo), NHALF(k_hi), 5]
        img16 = images[i].reshape(NPIX, K).astype(np.float16)
        v16 = np.maximum(
            uniform_noise[i].reshape(NPIX, K) - 1.0, np.float32(VMIN)
        ).astype(np.float16)
        in_maps.append(
            {
                "imgt": _khalf_blocks(img16),
                "vt": _khalf_blocks(v16),
                "pal": pal,
            }
        )
    return in_maps


def _unshard(results):
    outs = []
    for i in range(NCORES):
        o = np.asarray(results[i]["out"], dtype=np.float32)  # [NBLK,128,JB*4]
        # pixel = bi*F + j*128 + p
        o = o.reshape(NBLK, 128, JB, 4).transpose(0, 2, 1, 3).reshape(H, W, 4)
        outs.append(o)
    return np.stack(outs)  # [8, 256, 256, 4]


def kernel(**inputs) -> np.ndarray:
    from concourse.bass_utils import run_bass_kernel_spmd

    images = np.asarray(inputs["images"], dtype=np.float32)
    palettes = np.asarray(inputs["palettes"], dtype=np.float32)
    noise = np.asarray(inputs["uniform_noise"], dtype=np.float32)
    temp = float(np.asarray(inputs["temperature"]))

    nc = _get_nc(temp)
    in_maps = _make_in_maps(images, palettes, noise)
    res = run_bass_kernel_spmd(nc, in_maps, list(range(NCORES)))
    return _unshard(res.results)
